# revision 20
# baseline (speedup 1.0000x reference)
"""MLA (multi-latent-head attention) Trainium2 kernel.

Problem: B=2, T=2048, D_MODEL=2048, N_HEAD=16, D_C=512, D_HEAD=128, D_ROPE=32.

Sharding: 8 cores = 2 batches x 4 head-groups (4 heads per core).
Each core computes, for its batch b and heads [4g..4g+3]:
  c_kv^T = W_DKV^T x^T          [512, T]   (bf16, transposed orientation)
  c_q^T  = W_DQ^T  x^T          [512, T]
  kr^T   = rope(W_KR_g^T x^T)   [128, T]   (4 heads x 32 rope dims)
  qr^T   = rope(W_QR_g^T c_q^T) [128, T]
  kc^T_h = W_UK_h^T c_kv^T      [128, T] per head
  qc^T_h = W_UQ_h^T c_q^T       [128, T] per head
  V      = c_kv W_UV_g          [T, 512]  (natural orientation, 4 heads)
  S^T    = K Q^T  (per k-tile of 128, accumulated over d=128 content + d=32 rope)
  P^T    = exp(S^T / sqrt(160))           (no max subtraction; |S|<~2 by construction)
  OUT^T  = V^T P^T  (PSUM accum over k-tiles);  l = ones^T P^T  (softmax denom)
  OUT^T normalized by broadcast(1/l) and written [512, T] fp32; host transposes.

RoPE: weight columns pre-permuted on host to [even dims(16) | odd dims(16)] per
head; rope computed as pre*cos + swap(pre)*sin_signed where swap() is a
permutation matmul (16-row block swap within each 32-row head block) and
sin_signed carries the sign flip for the first half.

Scheduling notes:
- Projection loops run k-tile-outer with 4 PSUM chunk accumulators so the PE
  starts as soon as the first x tile lands and each weight tile is loaded once.
- Attention processes k-tiles in pairs with a [128, 1024] S/P tile: one
  ACTIVATE per pair (halves the ACT 352-cycle overheads) and fewer PSUM
  switches. The softmax denominator accumulates into row 32*qc of a single
  PSUM bank via the matmul tile_position output placement.
- PSUM pools are phase-stacked: phases 1-2 use one 6-buf accumulator pool;
  phase 3 uses S-pair(4 banks)/OUT(2)/l(1) plus the phase-1 pool for the
  1/l broadcast.
- SBUF is phased too: x^T tiles + streamed W_DKV/W_DQ blocks live only during
  phase 1; per-head K/Q/V reuse that space (stack-ordered tile pools).
"""

import sys

if "/opt/trn_rl_repo" not in sys.path:
    sys.path.insert(0, "/opt/trn_rl_repo")

import math

import ml_dtypes
import numpy as np

import concourse.bass as bass
import concourse.mybir as mybir
import concourse.tile as tile
from concourse.vector_clock import ScopedClock
from concourse.bass_utils import run_bass_kernel_spmd

BF16 = ml_dtypes.bfloat16

B, T, DM, NH, DC = 2, 2048, 2048, 16, 512
DH = DM // NH            # 128
DR = DC // NH            # 32
HL = 4                   # heads per core
D_ATT = DH + DR          # 160
SCALE = 1.0 / math.sqrt(D_ATT)

NKT_DM = DM // 128       # 16 k-tiles over d_model
NKT_DC = DC // 128       # 4  k-tiles over d_c
NTT = T // 128           # 16 tiles over T (k-tiles of attention)
NCH = T // 512           # 4  chunks of 512 over T
F32 = mybir.dt.float32
BF = mybir.dt.bfloat16


class PatchedTC(tile.TileContext):
    """This walrus build rejects >1 sync-wait on CTRL (Drain) instructions;
    split the final tile drain into one drain per semaphore wait."""

    def _drain_and_barrier(self, tick_clock, wait_clock):
        drain_inst = self.nc.sync.drain()
        wait_clock.add_sem_waits(
            drain_inst.ins, ScopedClock({None: tick_clock.global_clock})
        )
        si = drain_inst.ins.sync_info
        if si is not None and si.on_wait and len(si.on_wait) > 1:
            waits = list(si.on_wait)
            si.on_wait = waits[:1]
            for w in waits[1:]:
                d2 = self.nc.sync.drain()
                d2.ins.sync_info = mybir.SyncInfo(on_wait=[w], on_update=[])
        self.nc.all_engine_barrier()
        assert self.sems is not None
        popped = self.nc._tile_sem_poison_stack.pop()
        assert popped is self._sem_poison
        self.nc.clear_and_free_semaphores(list(self.sems.allocated().values()))
        self.nc.all_engine_barrier()


def _split_multi_waits(nc):
    """This walrus build rejects >1 sync-wait per instruction: move extra
    waits onto NoOp instructions inserted before the owner on its engine."""
    n = 0
    for fn in nc.m.functions:
        for bb in fn.blocks:
            out = []
            changed = False
            for inst in bb.instructions:
                si = inst.sync_info
                if si is not None and si.on_wait and len(si.on_wait) > 1:
                    waits = list(si.on_wait)
                    for w in waits[:-1]:
                        n += 1
                        nop = mybir.InstNoOp(
                            name=f"{inst.name}_w{n}", ins=[], outs=[],
                            sync_info=mybir.SyncInfo(on_wait=[w], on_update=[]),
                        )
                        nop.engine = inst.engine
                        out.append(nop)
                    si.on_wait = waits[-1:]
                    inst.sync_info = si
                    changed = True
                out.append(inst)
            if changed:
                bb.instructions = out


def _build_nc(with_biases):
    """Build the SPMD Bass program (identical on all cores; data differs)."""
    nc = bass.Bass()

    # ---- HBM inputs (per-core shards; layouts produced by host prep) ----
    xt = nc.dram_tensor("xt", [NKT_DM, 128, T], BF, kind="ExternalInput")
    # wdown: 9 column-blocks of [128, 16*128]: j=0..3 W_DKV, 4..7 W_DQ, 8 W_KR
    wdown = nc.dram_tensor("wdown", [9, 128, NKT_DM * 128], BF,
                           kind="ExternalInput")
    wuk = nc.dram_tensor("wuk", [128, NKT_DC * 512], BF, kind="ExternalInput")
    wuv = nc.dram_tensor("wuv", [128, NKT_DC * 512], BF, kind="ExternalInput")
    wuq = nc.dram_tensor("wuq", [128, NKT_DC * 512], BF, kind="ExternalInput")
    wqr = nc.dram_tensor("wqr", [128, NKT_DC * 128], BF, kind="ExternalInput")
    cos_d = nc.dram_tensor("cos", [128, T], F32, kind="ExternalInput")
    sin_d = nc.dram_tensor("sin", [128, T], F32, kind="ExternalInput")
    swp_d = nc.dram_tensor("swp", [128, 128], BF, kind="ExternalInput")
    ones_d = nc.dram_tensor("ones128", [128, 128], BF, kind="ExternalInput")
    if with_biases:
        # [128, 9]: cols 0-3 b_DKV tiles, 4-7 b_DQ tiles, 8 b_KR(g, permuted)
        bias1_d = nc.dram_tensor("bias1", [128, 9], F32, kind="ExternalInput")
        # [128, 9]: cols 0-3 b_UK(g), 4-7 b_UQ(g), 8 b_QR(g, permuted)
        bias2_d = nc.dram_tensor("bias2", [128, 9], F32, kind="ExternalInput")
        biasv_d = nc.dram_tensor("biasv", [128, 512], F32, kind="ExternalInput")

    out_t = nc.dram_tensor("out_t", [HL * 128, T], F32, kind="ExternalOutput")

    with PatchedTC(nc) as tc:
        # ---- persistent pools (bottom of the SBUF stack) ----
        with tc.tile_pool(name="consts", bufs=1) as consts, \
             tc.tile_pool(name="cpool", bufs=1) as cpool, \
             tc.tile_pool(name="ppool", bufs=3) as ppool, \
             tc.tile_pool(name="small", bufs=3) as small, \
             tc.tile_pool(name="opool", bufs=3) as opool:

            # ---- constants (tiles now; DMAs deferred until after the
            # x tiles + first weight block are queued, so phase 1 starts
            # as early as possible) ----
            w_uk = consts.tile([128, NKT_DC * 512], BF, tag="wuk")
            w_uv = consts.tile([128, NKT_DC * 512], BF, tag="wuv")
            w_uq = consts.tile([128, NKT_DC * 512], BF, tag="wuq")
            w_qr = consts.tile([128, NKT_DC * 128], BF, tag="wqr")
            cos_t = consts.tile([128, T], F32, tag="cos")
            sin_t = consts.tile([128, T], F32, tag="sin")
            swp_t = consts.tile([128, 128], BF, tag="swp")
            ones128 = consts.tile([128, 128], BF, tag="ones128")
            bias1 = bias2 = biasv = None
            if with_biases:
                bias1 = consts.tile([128, 9], F32, tag="bias1")
                bias2 = consts.tile([128, 9], F32, tag="bias2")
                biasv = consts.tile([128, 512], F32, tag="biasv")

            def dma_consts():
                nc.sync.dma_start(out=swp_t, in_=swp_d[:])
                nc.sync.dma_start(out=cos_t, in_=cos_d[:])
                nc.sync.dma_start(out=sin_t, in_=sin_d[:])
                nc.sync.dma_start(out=w_uk, in_=wuk[:])
                nc.sync.dma_start(out=w_uv, in_=wuv[:])
                nc.sync.dma_start(out=w_uq, in_=wuq[:])
                nc.sync.dma_start(out=w_qr, in_=wqr[:])
                nc.sync.dma_start(out=ones128, in_=ones_d[:])
                if with_biases:
                    nc.sync.dma_start(out=bias1, in_=bias1_d[:])
                    nc.sync.dma_start(out=bias2, in_=bias2_d[:])
                    nc.sync.dma_start(out=biasv, in_=biasv_d[:])

            # persistent phase-1 outputs
            ckv_t = [cpool.tile([128, T], BF, tag=f"ckv{j}", name=f"ckv{j}")
                     for j in range(4)]
            cq_t = [cpool.tile([128, T], BF, tag=f"cq{j}", name=f"cq{j}")
                    for j in range(4)]
            kr_pre = cpool.tile([128, T], BF, tag="kr_pre")
            kr_t = cpool.tile([128, T], BF, tag="kr")

            def evict(dst_ap, src_psum, bias_ap):
                if bias_ap is not None:
                    nc.vector.tensor_scalar_add(out=dst_ap, in0=src_psum,
                                                scalar1=bias_ap)
                else:
                    nc.vector.tensor_copy(out=dst_ap, in_=src_psum)

            # ===== phases 1-2: k-outer projections, 4 chunk accumulators ====
            # (PSUM pool closed before phase 3; PSUM/SBUF pool stacks are
            # independent, so this interleaves fine with the SBUF pools.)
            prps_cm = tc.tile_pool(name="prps", bufs=6, space="PSUM")
            prps = prps_cm.__enter__()
            if True:

                def proj(lhs_tiles, src_tiles, dst, b_ap, tag):
                    nkt = len(lhs_tiles)
                    pss = [prps.tile([128, 512], F32, tag="prps",
                                     name=f"{tag}ps{ch}") for ch in range(NCH)]
                    for kt in range(nkt):
                        for ch in range(NCH):
                            nc.tensor.matmul(
                                pss[ch],
                                lhsT=lhs_tiles[kt],
                                rhs=src_tiles[kt][:, ch * 512: (ch + 1) * 512],
                                start=(kt == 0),
                                stop=(kt == nkt - 1),
                            )
                    for ch in range(NCH):
                        evict(dst[:, ch * 512: (ch + 1) * 512], pss[ch], b_ap)

                def apply_rope(pre_tile, dsts):
                    """out = pre*cos + swap(pre)*sin_signed.
                    dsts: [(tile, row_slice)] destinations for the final add."""
                    for ch in range(NCH):
                        sl = slice(ch * 512, (ch + 1) * 512)
                        sw = prps.tile([128, 512], F32, tag="prps",
                                       name="swpsum")
                        nc.tensor.matmul(sw, lhsT=swp_t[:], rhs=pre_tile[:, sl],
                                         start=True, stop=True)
                        t1 = opool.tile([128, 512], F32, tag="rope_t1")
                        nc.vector.tensor_mul(out=t1, in0=pre_tile[:, sl],
                                             in1=cos_t[:, sl])
                        t2 = opool.tile([128, 512], F32, tag="rope_t2")
                        nc.vector.tensor_mul(out=t2, in0=sw, in1=sin_t[:, sl])
                        for dst, rp in dsts:
                            nc.vector.tensor_add(out=dst[rp, sl],
                                                 in0=t1[rp, :], in1=t2[rp, :])

                # ---- PHASE 1: x^T consumers (x + streamed W resident) ----
                with tc.tile_pool(name="xpool", bufs=1) as xpool, \
                     tc.tile_pool(name="wstream", bufs=2) as wstream:
                    wt0 = wstream.tile([128, NKT_DM * 128], BF, tag="wt",
                                       name="wt0")
                    nc.sync.dma_start(out=wt0, in_=wdown[0])
                    x_tiles = []
                    for kt in range(NKT_DM):
                        xtile = xpool.tile([128, T], BF, tag=f"x{kt}",
                                           name=f"x{kt}")
                        nc.sync.dma_start(out=xtile, in_=xt[kt])
                        x_tiles.append(xtile)
                    dma_consts()

                    for j in range(9):
                        if j == 0:
                            wt = wt0
                        else:
                            wt = wstream.tile([128, NKT_DM * 128], BF,
                                              tag="wt", name=f"wt{j}")
                            nc.sync.dma_start(out=wt, in_=wdown[j])
                        if j < 4:
                            dst = ckv_t[j]
                            b_ap = bias1[:, j:j + 1] if with_biases else None
                        elif j < 8:
                            dst = cq_t[j - 4]
                            b_ap = bias1[:, j:j + 1] if with_biases else None
                        else:
                            dst = kr_pre
                            b_ap = bias1[:, 8:9] if with_biases else None
                        proj([wt[:, kt * 128: (kt + 1) * 128]
                              for kt in range(NKT_DM)],
                             x_tiles, dst, b_ap, f"p1j{j}")

                apply_rope(kr_pre, [(kr_t, slice(0, 128))])

                # ---- PHASE 2+3: latent consumers (reuse x's SBUF) ----
                with tc.tile_pool(name="kqpool", bufs=1) as kqpool, \
                     tc.tile_pool(name="vpool", bufs=1) as vpool:
                    kc_t = [kqpool.tile([128, T], BF, tag=f"kc{h}",
                                        name=f"kc{h}") for h in range(HL)]
                    qc_t = [kqpool.tile([128, T], BF, tag=f"qc{h}",
                                        name=f"qc{h}") for h in range(HL)]
                    qr_pre = kqpool.tile([128, T], BF, tag="qr_pre")
                    # per-head zero-padded rope Q: only rows 32h..32h+32 live,
                    # so the rope S matmul is a plain full-128 (0,0) matmul
                    qr_pad = [kqpool.tile([128, T], BF, tag=f"qrp{h}",
                                          name=f"qrp{h}") for h in range(HL)]
                    for h in range(HL):
                        nc.gpsimd.memset(qr_pad[h][:], 0.0)

                    for h in range(HL):
                        proj([w_uk[:, kt * 512 + 128 * h:
                                   kt * 512 + 128 * (h + 1)]
                              for kt in range(NKT_DC)],
                             ckv_t, kc_t[h],
                             bias2[:, h:h + 1] if with_biases else None,
                             f"p2k{h}")
                    for h in range(HL):
                        proj([w_uq[:, kt * 512 + 128 * h:
                                   kt * 512 + 128 * (h + 1)]
                              for kt in range(NKT_DC)],
                             cq_t, qc_t[h],
                             bias2[:, 4 + h:5 + h] if with_biases else None,
                             f"p2q{h}")
                    proj([w_qr[:, kt * 128: (kt + 1) * 128]
                          for kt in range(NKT_DC)],
                         cq_t, qr_pre,
                         bias2[:, 8:9] if with_biases else None, "p2r")

                    apply_rope(qr_pre, [(qr_pad[h], slice(32 * h, 32 * h + 32))
                                        for h in range(HL)])

                    # V natural: [T-tile rows, 512 (4 heads x 128)]
                    v_nat = []
                    for tt in range(NTT):
                        ps = prps.tile([128, 512], F32, tag="prps",
                                       name=f"vps{tt}")
                        for kt in range(NKT_DC):
                            nc.tensor.matmul(
                                ps,
                                lhsT=ckv_t[kt][:, tt * 128: (tt + 1) * 128],
                                rhs=w_uv[:, kt * 512: (kt + 1) * 512],
                                start=(kt == 0),
                                stop=(kt == NKT_DC - 1),
                            )
                        vt = vpool.tile([128, 512], BF, tag=f"v{tt}",
                                        name=f"v{tt}")
                        if with_biases:
                            nc.vector.tensor_add(out=vt, in0=ps, in1=biasv)
                        else:
                            nc.vector.tensor_copy(out=vt, in_=ps)
                        v_nat.append(vt)

                    # ========== PHASE 3: attention ==========
                    prps_cm.__exit__(None, None, None)
                    with tc.tile_pool(name="sp2", bufs=2, space="PSUM") as sp2, \
                         tc.tile_pool(name="opp", bufs=2,
                                      space="PSUM") as op_pool, \
                         tc.tile_pool(name="lpp", bufs=2,
                                      space="PSUM") as lp_pool:
                        for h in range(HL):
                            for qc in range(NCH):
                                qsl = slice(qc * 512, (qc + 1) * 512)
                                # all-ones lhsT: every row of lacc = l[q]
                                # (full-128 matmul + free broadcast)
                                lacc = lp_pool.tile([128, 512], F32,
                                                    tag="lacc",
                                                    name=f"lacc{h}_{qc}")
                                outp = op_pool.tile([128, 512], F32,
                                                    tag="outp",
                                                    name=f"outp{h}_{qc}")
                                # software pipeline: PV/l of pair kp run
                                # after S of pair kp+1, so the PE never waits
                                # on the ACT exp.
                                pts = [None] * (NTT // 2)

                                def pv_l(kp):
                                    for ki in range(2):
                                        kt = 2 * kp + ki
                                        psl = slice(512 * ki, 512 * (ki + 1))
                                        nc.tensor.matmul(
                                            outp,
                                            lhsT=v_nat[kt][:, 128 * h:
                                                           128 * (h + 1)],
                                            rhs=pts[kp][:, psl],
                                            start=(kt == 0),
                                            stop=(kt == NTT - 1))
                                        nc.tensor.matmul(
                                            lacc, lhsT=ones128[:],
                                            rhs=pts[kp][:, psl],
                                            start=(kt == 0),
                                            stop=(kt == NTT - 1))

                                for kp in range(NTT // 2):
                                    spt = sp2.tile([128, 1024], F32, tag="sp",
                                                   name=f"sp{h}_{qc}_{kp}")
                                    for ki in range(2):
                                        kt = 2 * kp + ki
                                        ksl = slice(kt * 128, (kt + 1) * 128)
                                        half = spt[:, 512 * ki: 512 * (ki + 1)]
                                        nc.tensor.matmul(
                                            half, lhsT=kc_t[h][:, ksl],
                                            rhs=qc_t[h][:, qsl],
                                            start=True, stop=False)
                                        nc.tensor.matmul(
                                            half, lhsT=kr_t[:, ksl],
                                            rhs=qr_pad[h][:, qsl],
                                            start=False, stop=True)
                                    pt = ppool.tile([128, 1024], BF, tag="pt")
                                    nc.scalar.activation(
                                        out=pt, in_=spt,
                                        func=mybir.ActivationFunctionType.Exp,
                                        scale=SCALE)
                                    pts[kp] = pt
                                    if kp > 0:
                                        pv_l(kp - 1)
                                pv_l(NTT // 2 - 1)
                                # normalize: out = outp * (1/l)
                                rinv = small.tile([128, 512], F32, tag="rinv")
                                nc.vector.reciprocal(out=rinv, in_=lacc)
                                o_sb = opool.tile([128, 512], F32, tag="o_sb")
                                nc.vector.tensor_mul(out=o_sb, in0=outp,
                                                     in1=rinv)
                                nc.sync.dma_start(
                                    out=out_t[128 * h: 128 * (h + 1), qsl],
                                    in_=o_sb)

    _split_multi_waits(nc)
    return nc


_nc_cache = {}


def _get_nc(with_biases):
    if with_biases not in _nc_cache:
        _nc_cache[with_biases] = _build_nc(with_biases)
    return _nc_cache[with_biases]


def _rope_perm():
    """Permutation of the 32 rope dims within one head: evens then odds."""
    return np.concatenate([np.arange(0, DR, 2), np.arange(1, DR, 2)])


def kernel(x, W_DKV, b_DKV, W_UK, b_UK, W_UV, b_UV, W_DQ, b_DQ,
           W_UQ, b_UQ, W_QR, b_QR, W_KR, b_KR):
    x = np.asarray(x, np.float32)
    f32 = lambda a: np.asarray(a, np.float32)
    W_DKV, W_UK, W_UV, W_DQ, W_UQ, W_QR, W_KR = map(
        f32, (W_DKV, W_UK, W_UV, W_DQ, W_UQ, W_QR, W_KR))
    b_DKV, b_UK, b_UV, b_DQ, b_UQ, b_QR, b_KR = map(
        f32, (b_DKV, b_UK, b_UV, b_DQ, b_UQ, b_QR, b_KR))

    with_biases = any(np.any(b)
                      for b in (b_DKV, b_UK, b_UV, b_DQ, b_UQ, b_QR, b_KR))
    nc = _get_nc(with_biases)

    perm = _rope_perm()

    # lhsT-tile layout helper: W [K, C] -> [128, (K//128)*C], [p, kt*C + c]
    def tile_k(w):
        k, c = w.shape
        return np.ascontiguousarray(
            w.reshape(k // 128, 128, c).transpose(1, 0, 2).reshape(128, -1)
        ).astype(BF16)

    # column-block layout for streamed down-proj weights:
    # W [2048, C] -> per 128-col block j: [128, 16*128], [p, kt*128 + cc]
    def tile_k_blocks(w):
        k, c = w.shape
        nj = c // 128
        return np.ascontiguousarray(
            w.reshape(k // 128, 128, nj, 128).transpose(2, 1, 0, 3)
            .reshape(nj, 128, -1)
        ).astype(BF16)

    # x^T per batch, tiled over d_model: [16, 128, T]
    xt_b = []
    for b in range(B):
        xT = np.ascontiguousarray(x[b].T.astype(BF16))       # [DM, T]
        xt_b.append(np.ascontiguousarray(xT.reshape(NKT_DM, 128, T)))

    # RoPE tables: [128, T] fp32; rows 32h+i / 32h+16+i use freq i
    freqs = 10000.0 ** (-(np.arange(0, DR, 2, dtype=np.float64) / DR))   # [16]
    theta = np.arange(T, dtype=np.float64)[:, None] * freqs[None, :]     # [T, 16]
    cos16 = np.cos(theta).T.astype(np.float32)                           # [16, T]
    sin16 = np.sin(theta).T.astype(np.float32)
    cos_full = np.tile(cos16, (8, 1))                                    # [128, T]
    sin_signed = np.tile(np.concatenate([-sin16, sin16], 0), (4, 1))     # [128, T]

    # swap permutation matrix (16-row block swap inside each 32-row block)
    swp = np.zeros((128, 128), np.float32)
    for hb in range(4):
        for i in range(16):
            swp[32 * hb + 16 + i, 32 * hb + i] = 1.0
            swp[32 * hb + i, 32 * hb + 16 + i] = 1.0
    swp = swp.astype(BF16)

    ones128 = np.ones((128, 128), BF16)

    in_maps = []
    for c in range(8):
        b, g = divmod(c, 4)
        heads = slice(4 * g * DH, (4 * g + HL) * DH)          # content cols
        rcols = np.concatenate(
            [(4 * g + h) * DR + perm for h in range(HL)])     # rope cols
        wdown = np.concatenate([
            tile_k_blocks(W_DKV),            # j=0..3
            tile_k_blocks(W_DQ),             # j=4..7
            tile_k_blocks(W_KR[:, rcols]),   # j=8
        ], axis=0)
        RSTRIDE = 4 * 128 * 512
        JSTRIDE = 128 * 512
        m = {
            "xt": xt_b[b],
            "xs": np.ascontiguousarray(
                xt_b[b][:, :, 512 * g: 512 * (g + 1)]),
            "rbase": np.array([[4 * b * RSTRIDE + (j % 4) * JSTRIDE
                                for j in range(8)]], np.uint32),
            "wdown": wdown,
            "wuk": tile_k(W_UK[:, heads]),
            "wuv": tile_k(W_UV[:, heads]),
            "wuq": tile_k(W_UQ[:, heads]),
            "wqr": tile_k(W_QR[:, rcols]),
            "cos": cos_full,
            "sin": sin_signed,
            "swp": swp,
            "ones128": ones128,
        }
        if with_biases:
            bias1 = np.zeros((128, 9), np.float32)
            bias1[:, 0:4] = b_DKV.reshape(4, 128).T
            bias1[:, 4:8] = b_DQ.reshape(4, 128).T
            bias1[:, 8] = b_KR[rcols]
            bias2 = np.zeros((128, 9), np.float32)
            bias2[:, 0:4] = b_UK[heads].reshape(4, 128).T
            bias2[:, 4:8] = b_UQ[heads].reshape(4, 128).T
            bias2[:, 8] = b_QR[rcols]
            m["bias1"] = bias1
            m["bias2"] = bias2
            m["biasv"] = np.tile(b_UV[heads][None, :], (128, 1)).astype(np.float32)
        in_maps.append(m)

    res = run_bass_kernel_spmd(nc, in_maps, core_ids=list(range(8)))

    out = np.empty((B, T, DM), np.float32)
    for c in range(8):
        b, g = divmod(c, 4)
        ot = res.results[c]["out_t"]                    # [512, T]
        for h in range(HL):
            out[b, :, (4 * g + h) * DH: (4 * g + h + 1) * DH] = \
                ot[128 * h: 128 * (h + 1), :].T
    return out


# revision 21
# speedup vs baseline: 1.1013x; 1.1013x over previous
"""MLA (multi-latent-head attention) Trainium2 kernel.

Problem: B=2, T=2048, D_MODEL=2048, N_HEAD=16, D_C=512, D_HEAD=128, D_ROPE=32.

Sharding: 8 cores = 2 batches x 4 head-groups (4 heads per core).
Each core computes, for its batch b and heads [4g..4g+3]:
  c_kv^T = W_DKV^T x^T          [512, T]   (bf16, transposed orientation)
  c_q^T  = W_DQ^T  x^T          [512, T]
  kr^T   = rope(W_KR_g^T x^T)   [128, T]   (4 heads x 32 rope dims)
  qr^T   = rope(W_QR_g^T c_q^T) [128, T]
  kc^T_h = W_UK_h^T c_kv^T      [128, T] per head
  qc^T_h = W_UQ_h^T c_q^T       [128, T] per head
  V      = c_kv W_UV_g          [T, 512]  (natural orientation, 4 heads)
  S^T    = K Q^T  (per k-tile of 128, accumulated over d=128 content + d=32 rope)
  P^T    = exp(S^T / sqrt(160))           (no max subtraction; |S|<~2 by construction)
  OUT^T  = V^T P^T  (PSUM accum over k-tiles);  l = ones^T P^T  (softmax denom)
  OUT^T normalized by broadcast(1/l) and written [512, T] fp32; host transposes.

RoPE: weight columns pre-permuted on host to [even dims(16) | odd dims(16)] per
head; rope computed as pre*cos + swap(pre)*sin_signed where swap() is a
permutation matmul (16-row block swap within each 32-row head block) and
sin_signed carries the sign flip for the first half.

Scheduling notes:
- Projection loops run k-tile-outer with 4 PSUM chunk accumulators so the PE
  starts as soon as the first x tile lands and each weight tile is loaded once.
- Attention processes k-tiles in pairs with a [128, 1024] S/P tile: one
  ACTIVATE per pair (halves the ACT 352-cycle overheads) and fewer PSUM
  switches. The softmax denominator accumulates into row 32*qc of a single
  PSUM bank via the matmul tile_position output placement.
- PSUM pools are phase-stacked: phases 1-2 use one 6-buf accumulator pool;
  phase 3 uses S-pair(4 banks)/OUT(2)/l(1) plus the phase-1 pool for the
  1/l broadcast.
- SBUF is phased too: x^T tiles + streamed W_DKV/W_DQ blocks live only during
  phase 1; per-head K/Q/V reuse that space (stack-ordered tile pools).
"""

import sys

if "/opt/trn_rl_repo" not in sys.path:
    sys.path.insert(0, "/opt/trn_rl_repo")

import math

import ml_dtypes
import numpy as np

import concourse.bass as bass
import concourse.mybir as mybir
import concourse.tile as tile
from concourse.vector_clock import ScopedClock
from concourse.bass_utils import run_bass_kernel_spmd

BF16 = ml_dtypes.bfloat16

B, T, DM, NH, DC = 2, 2048, 2048, 16, 512
DH = DM // NH            # 128
DR = DC // NH            # 32
HL = 4                   # heads per core
D_ATT = DH + DR          # 160
SCALE = 1.0 / math.sqrt(D_ATT)

NKT_DM = DM // 128       # 16 k-tiles over d_model
NKT_DC = DC // 128       # 4  k-tiles over d_c
NTT = T // 128           # 16 tiles over T (k-tiles of attention)
NCH = T // 512           # 4  chunks of 512 over T
F32 = mybir.dt.float32
BF = mybir.dt.bfloat16


class PatchedTC(tile.TileContext):
    """This walrus build rejects >1 sync-wait on CTRL (Drain) instructions;
    split the final tile drain into one drain per semaphore wait."""

    def _drain_and_barrier(self, tick_clock, wait_clock):
        drain_inst = self.nc.sync.drain()
        wait_clock.add_sem_waits(
            drain_inst.ins, ScopedClock({None: tick_clock.global_clock})
        )
        si = drain_inst.ins.sync_info
        if si is not None and si.on_wait and len(si.on_wait) > 1:
            waits = list(si.on_wait)
            si.on_wait = waits[:1]
            for w in waits[1:]:
                d2 = self.nc.sync.drain()
                d2.ins.sync_info = mybir.SyncInfo(on_wait=[w], on_update=[])
        self.nc.all_engine_barrier()
        assert self.sems is not None
        popped = self.nc._tile_sem_poison_stack.pop()
        assert popped is self._sem_poison
        self.nc.clear_and_free_semaphores(list(self.sems.allocated().values()))
        self.nc.all_engine_barrier()


def _split_multi_waits(nc):
    """This walrus build rejects >1 sync-wait per instruction: move extra
    waits onto NoOp instructions inserted before the owner on its engine."""
    n = 0
    for fn in nc.m.functions:
        for bb in fn.blocks:
            out = []
            changed = False
            for inst in bb.instructions:
                si = inst.sync_info
                if si is not None and si.on_wait and len(si.on_wait) > 1:
                    waits = list(si.on_wait)
                    for w in waits[:-1]:
                        n += 1
                        nop = mybir.InstNoOp(
                            name=f"{inst.name}_w{n}", ins=[], outs=[],
                            sync_info=mybir.SyncInfo(on_wait=[w], on_update=[]),
                        )
                        nop.engine = inst.engine
                        out.append(nop)
                    si.on_wait = waits[-1:]
                    inst.sync_info = si
                    changed = True
                out.append(inst)
            if changed:
                bb.instructions = out


def _build_nc(with_biases):
    """Build the SPMD Bass program (identical on all cores; data differs)."""
    nc = bass.Bass()

    # ---- HBM inputs (per-core shards; layouts produced by host prep) ----
    xt = nc.dram_tensor("xt", [NKT_DM, 128, T], BF, kind="ExternalInput")
    # wdown: 9 column-blocks of [128, 16*128]: j=0..3 W_DKV, 4..7 W_DQ, 8 W_KR
    wdown = nc.dram_tensor("wdown", [9, 128, NKT_DM * 128], BF,
                           kind="ExternalInput")
    wuk = nc.dram_tensor("wuk", [128, NKT_DC * 512], BF, kind="ExternalInput")
    wuv = nc.dram_tensor("wuv", [128, NKT_DC * 512], BF, kind="ExternalInput")
    wuq = nc.dram_tensor("wuq", [128, NKT_DC * 512], BF, kind="ExternalInput")
    wqr = nc.dram_tensor("wqr", [128, NKT_DC * 128], BF, kind="ExternalInput")
    cos_d = nc.dram_tensor("cos", [128, T], F32, kind="ExternalInput")
    sin_d = nc.dram_tensor("sin", [128, T], F32, kind="ExternalInput")
    swp_d = nc.dram_tensor("swp", [128, 128], BF, kind="ExternalInput")
    ones_d = nc.dram_tensor("ones128", [128, 128], BF, kind="ExternalInput")
    if with_biases:
        # [128, 9]: cols 0-3 b_DKV tiles, 4-7 b_DQ tiles, 8 b_KR(g, permuted)
        bias1_d = nc.dram_tensor("bias1", [128, 9], F32, kind="ExternalInput")
        # [128, 9]: cols 0-3 b_UK(g), 4-7 b_UQ(g), 8 b_QR(g, permuted)
        bias2_d = nc.dram_tensor("bias2", [128, 9], F32, kind="ExternalInput")
        biasv_d = nc.dram_tensor("biasv", [128, 512], F32, kind="ExternalInput")

    out_t = nc.dram_tensor("out_t", [HL * 128, T], F32, kind="ExternalOutput")

    with PatchedTC(nc) as tc:
        # ---- persistent pools (bottom of the SBUF stack) ----
        with tc.tile_pool(name="consts", bufs=1) as consts, \
             tc.tile_pool(name="cpool", bufs=1) as cpool, \
             tc.tile_pool(name="ppool", bufs=3) as ppool, \
             tc.tile_pool(name="small", bufs=3) as small, \
             tc.tile_pool(name="opool", bufs=3) as opool:

            # ---- constants (tiles now; DMAs deferred until after the
            # x tiles + first weight block are queued, so phase 1 starts
            # as early as possible) ----
            w_uk = consts.tile([128, NKT_DC * 512], BF, tag="wuk")
            w_uv = consts.tile([128, NKT_DC * 512], BF, tag="wuv")
            w_uq = consts.tile([128, NKT_DC * 512], BF, tag="wuq")
            w_qr = consts.tile([128, NKT_DC * 128], BF, tag="wqr")
            cos_t = consts.tile([128, T], F32, tag="cos")
            sin_t = consts.tile([128, T], F32, tag="sin")
            swp_t = consts.tile([128, 128], BF, tag="swp")
            ones128 = consts.tile([128, 128], BF, tag="ones128")
            bias1 = bias2 = biasv = None
            if with_biases:
                bias1 = consts.tile([128, 9], F32, tag="bias1")
                bias2 = consts.tile([128, 9], F32, tag="bias2")
                biasv = consts.tile([128, 512], F32, tag="biasv")

            def dma_consts():
                nc.sync.dma_start(out=swp_t, in_=swp_d[:])
                nc.sync.dma_start(out=cos_t, in_=cos_d[:])
                nc.sync.dma_start(out=sin_t, in_=sin_d[:])
                nc.sync.dma_start(out=w_uk, in_=wuk[:])
                nc.sync.dma_start(out=w_uv, in_=wuv[:])
                nc.sync.dma_start(out=w_uq, in_=wuq[:])
                nc.sync.dma_start(out=w_qr, in_=wqr[:])
                nc.sync.dma_start(out=ones128, in_=ones_d[:])
                if with_biases:
                    nc.sync.dma_start(out=bias1, in_=bias1_d[:])
                    nc.sync.dma_start(out=bias2, in_=bias2_d[:])
                    nc.sync.dma_start(out=biasv, in_=biasv_d[:])

            # persistent phase-1 outputs
            ckv_t = [cpool.tile([128, T], BF, tag=f"ckv{j}", name=f"ckv{j}")
                     for j in range(4)]
            cq_t = [cpool.tile([128, T], BF, tag=f"cq{j}", name=f"cq{j}")
                    for j in range(4)]
            kr_pre = cpool.tile([128, T], BF, tag="kr_pre")
            kr_t = cpool.tile([128, T], BF, tag="kr")

            def evict(dst_ap, src_psum, bias_ap):
                if bias_ap is not None:
                    nc.vector.tensor_scalar_add(out=dst_ap, in0=src_psum,
                                                scalar1=bias_ap)
                else:
                    nc.vector.tensor_copy(out=dst_ap, in_=src_psum)

            # ===== phases 1-2: k-outer projections, 4 chunk accumulators ====
            # (PSUM pool closed before phase 3; PSUM/SBUF pool stacks are
            # independent, so this interleaves fine with the SBUF pools.)
            prps_cm = tc.tile_pool(name="prps", bufs=6, space="PSUM")
            prps = prps_cm.__enter__()
            if True:

                def proj(lhs_tiles, src_tiles, dst, b_ap, tag):
                    nkt = len(lhs_tiles)
                    pss = [prps.tile([128, 512], F32, tag="prps",
                                     name=f"{tag}ps{ch}") for ch in range(NCH)]
                    for kt in range(nkt):
                        for ch in range(NCH):
                            nc.tensor.matmul(
                                pss[ch],
                                lhsT=lhs_tiles[kt],
                                rhs=src_tiles[kt][:, ch * 512: (ch + 1) * 512],
                                start=(kt == 0),
                                stop=(kt == nkt - 1),
                            )
                    for ch in range(NCH):
                        evict(dst[:, ch * 512: (ch + 1) * 512], pss[ch], b_ap)

                def apply_rope(pre_tile, dsts):
                    """out = pre*cos + swap(pre)*sin_signed.
                    dsts: [(tile, row_slice)] destinations for the final add."""
                    for ch in range(NCH):
                        sl = slice(ch * 512, (ch + 1) * 512)
                        sw = prps.tile([128, 512], F32, tag="prps",
                                       name="swpsum")
                        nc.tensor.matmul(sw, lhsT=swp_t[:], rhs=pre_tile[:, sl],
                                         start=True, stop=True)
                        t1 = opool.tile([128, 512], F32, tag="rope_t1")
                        nc.vector.tensor_mul(out=t1, in0=pre_tile[:, sl],
                                             in1=cos_t[:, sl])
                        t2 = opool.tile([128, 512], F32, tag="rope_t2")
                        nc.vector.tensor_mul(out=t2, in0=sw, in1=sin_t[:, sl])
                        for dst, rp in dsts:
                            nc.vector.tensor_add(out=dst[rp, sl],
                                                 in0=t1[rp, :], in1=t2[rp, :])

                # ---- PHASE 1: x^T consumers (x + streamed W resident) ----
                with tc.tile_pool(name="xpool", bufs=1) as xpool, \
                     tc.tile_pool(name="wstream", bufs=2) as wstream:
                    wt0 = wstream.tile([128, NKT_DM * 128], BF, tag="wt",
                                       name="wt0")
                    nc.scalar.dma_start(out=wt0, in_=wdown[0])
                    x_tiles = []
                    for kt in range(NKT_DM):
                        xtile = xpool.tile([128, T], BF, tag=f"x{kt}",
                                           name=f"x{kt}")
                        nc.sync.dma_start(out=xtile, in_=xt[kt])
                        x_tiles.append(xtile)
                    dma_consts()

                    for j in range(9):
                        if j == 0:
                            wt = wt0
                        else:
                            wt = wstream.tile([128, NKT_DM * 128], BF,
                                              tag="wt", name=f"wt{j}")
                            nc.scalar.dma_start(out=wt, in_=wdown[j])
                        if j < 4:
                            dst = ckv_t[j]
                            b_ap = bias1[:, j:j + 1] if with_biases else None
                        elif j < 8:
                            dst = cq_t[j - 4]
                            b_ap = bias1[:, j:j + 1] if with_biases else None
                        else:
                            dst = kr_pre
                            b_ap = bias1[:, 8:9] if with_biases else None
                        proj([wt[:, kt * 128: (kt + 1) * 128]
                              for kt in range(NKT_DM)],
                             x_tiles, dst, b_ap, f"p1j{j}")

                apply_rope(kr_pre, [(kr_t, slice(0, 128))])

                # ---- PHASE 2+3: latent consumers (reuse x's SBUF) ----
                with tc.tile_pool(name="kqpool", bufs=1) as kqpool, \
                     tc.tile_pool(name="vpool", bufs=1) as vpool:
                    kc_t = [kqpool.tile([128, T], BF, tag=f"kc{h}",
                                        name=f"kc{h}") for h in range(HL)]
                    qc_t = [kqpool.tile([128, T], BF, tag=f"qc{h}",
                                        name=f"qc{h}") for h in range(HL)]
                    qr_pre = kqpool.tile([128, T], BF, tag="qr_pre")
                    # per-head zero-padded rope Q: only rows 32h..32h+32 live,
                    # so the rope S matmul is a plain full-128 (0,0) matmul
                    qr_pad = [kqpool.tile([128, T], BF, tag=f"qrp{h}",
                                          name=f"qrp{h}") for h in range(HL)]
                    for h in range(HL):
                        nc.gpsimd.memset(qr_pad[h][:], 0.0)

                    for h in range(HL):
                        proj([w_uk[:, kt * 512 + 128 * h:
                                   kt * 512 + 128 * (h + 1)]
                              for kt in range(NKT_DC)],
                             ckv_t, kc_t[h],
                             bias2[:, h:h + 1] if with_biases else None,
                             f"p2k{h}")
                    for h in range(HL):
                        proj([w_uq[:, kt * 512 + 128 * h:
                                   kt * 512 + 128 * (h + 1)]
                              for kt in range(NKT_DC)],
                             cq_t, qc_t[h],
                             bias2[:, 4 + h:5 + h] if with_biases else None,
                             f"p2q{h}")
                    proj([w_qr[:, kt * 128: (kt + 1) * 128]
                          for kt in range(NKT_DC)],
                         cq_t, qr_pre,
                         bias2[:, 8:9] if with_biases else None, "p2r")

                    apply_rope(qr_pre, [(qr_pad[h], slice(32 * h, 32 * h + 32))
                                        for h in range(HL)])

                    # V natural: [T-tile rows, 512 (4 heads x 128)]
                    v_nat = []
                    for tt in range(NTT):
                        ps = prps.tile([128, 512], F32, tag="prps",
                                       name=f"vps{tt}")
                        for kt in range(NKT_DC):
                            nc.tensor.matmul(
                                ps,
                                lhsT=ckv_t[kt][:, tt * 128: (tt + 1) * 128],
                                rhs=w_uv[:, kt * 512: (kt + 1) * 512],
                                start=(kt == 0),
                                stop=(kt == NKT_DC - 1),
                            )
                        vt = vpool.tile([128, 512], BF, tag=f"v{tt}",
                                        name=f"v{tt}")
                        if with_biases:
                            nc.vector.tensor_add(out=vt, in0=ps, in1=biasv)
                        else:
                            nc.vector.tensor_copy(out=vt, in_=ps)
                        v_nat.append(vt)

                    # ========== PHASE 3: attention ==========
                    prps_cm.__exit__(None, None, None)
                    with tc.tile_pool(name="sp2", bufs=2, space="PSUM") as sp2, \
                         tc.tile_pool(name="opp", bufs=2,
                                      space="PSUM") as op_pool, \
                         tc.tile_pool(name="lpp", bufs=2,
                                      space="PSUM") as lp_pool:
                        for h in range(HL):
                            for qc in range(NCH):
                                qsl = slice(qc * 512, (qc + 1) * 512)
                                # all-ones lhsT: every row of lacc = l[q]
                                # (full-128 matmul + free broadcast)
                                lacc = lp_pool.tile([128, 512], F32,
                                                    tag="lacc",
                                                    name=f"lacc{h}_{qc}")
                                outp = op_pool.tile([128, 512], F32,
                                                    tag="outp",
                                                    name=f"outp{h}_{qc}")
                                # software pipeline: PV/l of pair kp run
                                # after S of pair kp+1, so the PE never waits
                                # on the ACT exp.
                                pts = [None] * (NTT // 2)

                                def pv_l(kp):
                                    for ki in range(2):
                                        kt = 2 * kp + ki
                                        psl = slice(512 * ki, 512 * (ki + 1))
                                        nc.tensor.matmul(
                                            outp,
                                            lhsT=v_nat[kt][:, 128 * h:
                                                           128 * (h + 1)],
                                            rhs=pts[kp][:, psl],
                                            start=(kt == 0),
                                            stop=(kt == NTT - 1))
                                        nc.tensor.matmul(
                                            lacc, lhsT=ones128[:],
                                            rhs=pts[kp][:, psl],
                                            start=(kt == 0),
                                            stop=(kt == NTT - 1))

                                for kp in range(NTT // 2):
                                    spt = sp2.tile([128, 1024], F32, tag="sp",
                                                   name=f"sp{h}_{qc}_{kp}")
                                    for ki in range(2):
                                        kt = 2 * kp + ki
                                        ksl = slice(kt * 128, (kt + 1) * 128)
                                        half = spt[:, 512 * ki: 512 * (ki + 1)]
                                        nc.tensor.matmul(
                                            half, lhsT=kc_t[h][:, ksl],
                                            rhs=qc_t[h][:, qsl],
                                            start=True, stop=False)
                                        nc.tensor.matmul(
                                            half, lhsT=kr_t[:, ksl],
                                            rhs=qr_pad[h][:, qsl],
                                            start=False, stop=True)
                                    pt = ppool.tile([128, 1024], BF, tag="pt")
                                    nc.scalar.activation(
                                        out=pt, in_=spt,
                                        func=mybir.ActivationFunctionType.Exp,
                                        scale=SCALE)
                                    pts[kp] = pt
                                    if kp > 0:
                                        pv_l(kp - 1)
                                pv_l(NTT // 2 - 1)
                                # normalize: out = outp * (1/l)
                                rinv = small.tile([128, 512], F32, tag="rinv")
                                nc.vector.reciprocal(out=rinv, in_=lacc)
                                o_sb = opool.tile([128, 512], F32, tag="o_sb")
                                nc.vector.tensor_mul(out=o_sb, in0=outp,
                                                     in1=rinv)
                                nc.sync.dma_start(
                                    out=out_t[128 * h: 128 * (h + 1), qsl],
                                    in_=o_sb)

    _split_multi_waits(nc)
    return nc


_nc_cache = {}


def _get_nc(with_biases):
    if with_biases not in _nc_cache:
        _nc_cache[with_biases] = _build_nc(with_biases)
    return _nc_cache[with_biases]


def _rope_perm():
    """Permutation of the 32 rope dims within one head: evens then odds."""
    return np.concatenate([np.arange(0, DR, 2), np.arange(1, DR, 2)])


def kernel(x, W_DKV, b_DKV, W_UK, b_UK, W_UV, b_UV, W_DQ, b_DQ,
           W_UQ, b_UQ, W_QR, b_QR, W_KR, b_KR):
    x = np.asarray(x, np.float32)
    f32 = lambda a: np.asarray(a, np.float32)
    W_DKV, W_UK, W_UV, W_DQ, W_UQ, W_QR, W_KR = map(
        f32, (W_DKV, W_UK, W_UV, W_DQ, W_UQ, W_QR, W_KR))
    b_DKV, b_UK, b_UV, b_DQ, b_UQ, b_QR, b_KR = map(
        f32, (b_DKV, b_UK, b_UV, b_DQ, b_UQ, b_QR, b_KR))

    with_biases = any(np.any(b)
                      for b in (b_DKV, b_UK, b_UV, b_DQ, b_UQ, b_QR, b_KR))
    nc = _get_nc(with_biases)

    perm = _rope_perm()

    # lhsT-tile layout helper: W [K, C] -> [128, (K//128)*C], [p, kt*C + c]
    def tile_k(w):
        k, c = w.shape
        return np.ascontiguousarray(
            w.reshape(k // 128, 128, c).transpose(1, 0, 2).reshape(128, -1)
        ).astype(BF16)

    # column-block layout for streamed down-proj weights:
    # W [2048, C] -> per 128-col block j: [128, 16*128], [p, kt*128 + cc]
    def tile_k_blocks(w):
        k, c = w.shape
        nj = c // 128
        return np.ascontiguousarray(
            w.reshape(k // 128, 128, nj, 128).transpose(2, 1, 0, 3)
            .reshape(nj, 128, -1)
        ).astype(BF16)

    # x^T per batch, tiled over d_model: [16, 128, T]
    xt_b = []
    for b in range(B):
        xT = np.ascontiguousarray(x[b].T.astype(BF16))       # [DM, T]
        xt_b.append(np.ascontiguousarray(xT.reshape(NKT_DM, 128, T)))

    # RoPE tables: [128, T] fp32; rows 32h+i / 32h+16+i use freq i
    freqs = 10000.0 ** (-(np.arange(0, DR, 2, dtype=np.float64) / DR))   # [16]
    theta = np.arange(T, dtype=np.float64)[:, None] * freqs[None, :]     # [T, 16]
    cos16 = np.cos(theta).T.astype(np.float32)                           # [16, T]
    sin16 = np.sin(theta).T.astype(np.float32)
    cos_full = np.tile(cos16, (8, 1))                                    # [128, T]
    sin_signed = np.tile(np.concatenate([-sin16, sin16], 0), (4, 1))     # [128, T]

    # swap permutation matrix (16-row block swap inside each 32-row block)
    swp = np.zeros((128, 128), np.float32)
    for hb in range(4):
        for i in range(16):
            swp[32 * hb + 16 + i, 32 * hb + i] = 1.0
            swp[32 * hb + i, 32 * hb + 16 + i] = 1.0
    swp = swp.astype(BF16)

    ones128 = np.ones((128, 128), BF16)

    in_maps = []
    for c in range(8):
        b, g = divmod(c, 4)
        heads = slice(4 * g * DH, (4 * g + HL) * DH)          # content cols
        rcols = np.concatenate(
            [(4 * g + h) * DR + perm for h in range(HL)])     # rope cols
        wdown = np.concatenate([
            tile_k_blocks(W_DKV),            # j=0..3
            tile_k_blocks(W_DQ),             # j=4..7
            tile_k_blocks(W_KR[:, rcols]),   # j=8
        ], axis=0)
        RSTRIDE = 4 * 128 * 512
        JSTRIDE = 128 * 512
        m = {
            "xt": xt_b[b],
            "xs": np.ascontiguousarray(
                xt_b[b][:, :, 512 * g: 512 * (g + 1)]),
            "rbase": np.array([[4 * b * RSTRIDE + (j % 4) * JSTRIDE
                                for j in range(8)]], np.uint32),
            "wdown": wdown,
            "wuk": tile_k(W_UK[:, heads]),
            "wuv": tile_k(W_UV[:, heads]),
            "wuq": tile_k(W_UQ[:, heads]),
            "wqr": tile_k(W_QR[:, rcols]),
            "cos": cos_full,
            "sin": sin_signed,
            "swp": swp,
            "ones128": ones128,
        }
        if with_biases:
            bias1 = np.zeros((128, 9), np.float32)
            bias1[:, 0:4] = b_DKV.reshape(4, 128).T
            bias1[:, 4:8] = b_DQ.reshape(4, 128).T
            bias1[:, 8] = b_KR[rcols]
            bias2 = np.zeros((128, 9), np.float32)
            bias2[:, 0:4] = b_UK[heads].reshape(4, 128).T
            bias2[:, 4:8] = b_UQ[heads].reshape(4, 128).T
            bias2[:, 8] = b_QR[rcols]
            m["bias1"] = bias1
            m["bias2"] = bias2
            m["biasv"] = np.tile(b_UV[heads][None, :], (128, 1)).astype(np.float32)
        in_maps.append(m)

    res = run_bass_kernel_spmd(nc, in_maps, core_ids=list(range(8)))

    out = np.empty((B, T, DM), np.float32)
    for c in range(8):
        b, g = divmod(c, 4)
        ot = res.results[c]["out_t"]                    # [512, T]
        for h in range(HL):
            out[b, :, (4 * g + h) * DH: (4 * g + h + 1) * DH] = \
                ot[128 * h: 128 * (h + 1), :].T
    return out


# revision 25
# speedup vs baseline: 1.1051x; 1.0034x over previous
"""MLA (multi-latent-head attention) Trainium2 kernel.

Problem: B=2, T=2048, D_MODEL=2048, N_HEAD=16, D_C=512, D_HEAD=128, D_ROPE=32.

Sharding: 8 cores = 2 batches x 4 head-groups (4 heads per core).
Each core computes, for its batch b and heads [4g..4g+3]:
  c_kv^T = W_DKV^T x^T          [512, T]   (bf16, transposed orientation)
  c_q^T  = W_DQ^T  x^T          [512, T]
  kr^T   = rope(W_KR_g^T x^T)   [128, T]   (4 heads x 32 rope dims)
  qr^T   = rope(W_QR_g^T c_q^T) [128, T]
  kc^T_h = W_UK_h^T c_kv^T      [128, T] per head
  qc^T_h = W_UQ_h^T c_q^T       [128, T] per head
  V      = c_kv W_UV_g          [T, 512]  (natural orientation, 4 heads)
  S^T    = K Q^T  (per k-tile of 128, accumulated over d=128 content + d=32 rope)
  P^T    = exp(S^T / sqrt(160))           (no max subtraction; |S|<~2 by construction)
  OUT^T  = V^T P^T  (PSUM accum over k-tiles);  l = ones^T P^T  (softmax denom)
  OUT^T normalized by broadcast(1/l) and written [512, T] fp32; host transposes.

RoPE: weight columns pre-permuted on host to [even dims(16) | odd dims(16)] per
head; rope computed as pre*cos + swap(pre)*sin_signed where swap() is a
permutation matmul (16-row block swap within each 32-row head block) and
sin_signed carries the sign flip for the first half.

Scheduling notes:
- Projection loops run k-tile-outer with 4 PSUM chunk accumulators so the PE
  starts as soon as the first x tile lands and each weight tile is loaded once.
- Attention processes k-tiles in pairs with a [128, 1024] S/P tile: one
  ACTIVATE per pair (halves the ACT 352-cycle overheads) and fewer PSUM
  switches. The softmax denominator accumulates into row 32*qc of a single
  PSUM bank via the matmul tile_position output placement.
- PSUM pools are phase-stacked: phases 1-2 use one 6-buf accumulator pool;
  phase 3 uses S-pair(4 banks)/OUT(2)/l(1) plus the phase-1 pool for the
  1/l broadcast.
- SBUF is phased too: x^T tiles + streamed W_DKV/W_DQ blocks live only during
  phase 1; per-head K/Q/V reuse that space (stack-ordered tile pools).
"""

import sys

if "/opt/trn_rl_repo" not in sys.path:
    sys.path.insert(0, "/opt/trn_rl_repo")

import math

import ml_dtypes
import numpy as np

import concourse.bass as bass
import concourse.mybir as mybir
import concourse.tile as tile
from concourse.vector_clock import ScopedClock
from concourse.bass_utils import run_bass_kernel_spmd

BF16 = ml_dtypes.bfloat16

B, T, DM, NH, DC = 2, 2048, 2048, 16, 512
DH = DM // NH            # 128
DR = DC // NH            # 32
HL = 4                   # heads per core
D_ATT = DH + DR          # 160
SCALE = 1.0 / math.sqrt(D_ATT)

NKT_DM = DM // 128       # 16 k-tiles over d_model
NKT_DC = DC // 128       # 4  k-tiles over d_c
NTT = T // 128           # 16 tiles over T (k-tiles of attention)
NCH = T // 512           # 4  chunks of 512 over T
F32 = mybir.dt.float32
BF = mybir.dt.bfloat16


class PatchedTC(tile.TileContext):
    """This walrus build rejects >1 sync-wait on CTRL (Drain) instructions;
    split the final tile drain into one drain per semaphore wait."""

    def _drain_and_barrier(self, tick_clock, wait_clock):
        drain_inst = self.nc.sync.drain()
        wait_clock.add_sem_waits(
            drain_inst.ins, ScopedClock({None: tick_clock.global_clock})
        )
        si = drain_inst.ins.sync_info
        if si is not None and si.on_wait and len(si.on_wait) > 1:
            waits = list(si.on_wait)
            si.on_wait = waits[:1]
            for w in waits[1:]:
                d2 = self.nc.sync.drain()
                d2.ins.sync_info = mybir.SyncInfo(on_wait=[w], on_update=[])
        self.nc.all_engine_barrier()
        assert self.sems is not None
        popped = self.nc._tile_sem_poison_stack.pop()
        assert popped is self._sem_poison
        self.nc.clear_and_free_semaphores(list(self.sems.allocated().values()))
        self.nc.all_engine_barrier()


def _split_multi_waits(nc):
    """This walrus build rejects >1 sync-wait per instruction: move extra
    waits onto NoOp instructions inserted before the owner on its engine."""
    n = 0
    for fn in nc.m.functions:
        for bb in fn.blocks:
            out = []
            changed = False
            for inst in bb.instructions:
                si = inst.sync_info
                if si is not None and si.on_wait and len(si.on_wait) > 1:
                    waits = list(si.on_wait)
                    for w in waits[:-1]:
                        n += 1
                        nop = mybir.InstNoOp(
                            name=f"{inst.name}_w{n}", ins=[], outs=[],
                            sync_info=mybir.SyncInfo(on_wait=[w], on_update=[]),
                        )
                        nop.engine = inst.engine
                        out.append(nop)
                    si.on_wait = waits[-1:]
                    inst.sync_info = si
                    changed = True
                out.append(inst)
            if changed:
                bb.instructions = out


def _build_nc(with_biases):
    """Build the SPMD Bass program (identical on all cores; data differs)."""
    nc = bass.Bass()

    # ---- HBM inputs (per-core shards; layouts produced by host prep) ----
    xt = nc.dram_tensor("xt", [NKT_DM, 128, T], BF, kind="ExternalInput")
    # wdown: 9 column-blocks of [128, 16*128]: j=0..3 W_DKV, 4..7 W_DQ, 8 W_KR
    wdown = nc.dram_tensor("wdown", [9, 128, NKT_DM * 128], BF,
                           kind="ExternalInput")
    wuk = nc.dram_tensor("wuk", [128, NKT_DC * 512], BF, kind="ExternalInput")
    wuv = nc.dram_tensor("wuv", [128, NKT_DC * 512], BF, kind="ExternalInput")
    wuq = nc.dram_tensor("wuq", [128, NKT_DC * 512], BF, kind="ExternalInput")
    wqr = nc.dram_tensor("wqr", [128, NKT_DC * 128], BF, kind="ExternalInput")
    cos_d = nc.dram_tensor("cos", [128, T], F32, kind="ExternalInput")
    sin_d = nc.dram_tensor("sin", [128, T], F32, kind="ExternalInput")
    swp_d = nc.dram_tensor("swp", [128, 128], BF, kind="ExternalInput")
    ones_d = nc.dram_tensor("ones128", [128, 128], BF, kind="ExternalInput")
    if with_biases:
        # [128, 9]: cols 0-3 b_DKV tiles, 4-7 b_DQ tiles, 8 b_KR(g, permuted)
        bias1_d = nc.dram_tensor("bias1", [128, 9], F32, kind="ExternalInput")
        # [128, 9]: cols 0-3 b_UK(g), 4-7 b_UQ(g), 8 b_QR(g, permuted)
        bias2_d = nc.dram_tensor("bias2", [128, 9], F32, kind="ExternalInput")
        biasv_d = nc.dram_tensor("biasv", [128, 512], F32, kind="ExternalInput")

    out_t = nc.dram_tensor("out_t", [HL * 128, T], F32, kind="ExternalOutput")

    with PatchedTC(nc) as tc:
        # ---- persistent pools (bottom of the SBUF stack) ----
        with tc.tile_pool(name="consts", bufs=1) as consts, \
             tc.tile_pool(name="cpool", bufs=1) as cpool, \
             tc.tile_pool(name="ppool", bufs=4) as ppool, \
             tc.tile_pool(name="small", bufs=3) as small, \
             tc.tile_pool(name="opool", bufs=3) as opool:

            # ---- constants (tiles now; DMAs deferred until after the
            # x tiles + first weight block are queued, so phase 1 starts
            # as early as possible) ----
            w_uk = consts.tile([128, NKT_DC * 512], BF, tag="wuk")
            w_uv = consts.tile([128, NKT_DC * 512], BF, tag="wuv")
            w_uq = consts.tile([128, NKT_DC * 512], BF, tag="wuq")
            w_qr = consts.tile([128, NKT_DC * 128], BF, tag="wqr")
            cos_t = consts.tile([128, T], F32, tag="cos")
            sin_t = consts.tile([128, T], F32, tag="sin")
            swp_t = consts.tile([128, 128], BF, tag="swp")
            ones128 = consts.tile([128, 128], BF, tag="ones128")
            bias1 = bias2 = biasv = None
            if with_biases:
                bias1 = consts.tile([128, 9], F32, tag="bias1")
                bias2 = consts.tile([128, 9], F32, tag="bias2")
                biasv = consts.tile([128, 512], F32, tag="biasv")

            def dma_consts():
                nc.sync.dma_start(out=swp_t, in_=swp_d[:])
                nc.sync.dma_start(out=cos_t, in_=cos_d[:])
                nc.sync.dma_start(out=sin_t, in_=sin_d[:])
                nc.sync.dma_start(out=w_uk, in_=wuk[:])
                nc.sync.dma_start(out=w_uv, in_=wuv[:])
                nc.sync.dma_start(out=w_uq, in_=wuq[:])
                nc.sync.dma_start(out=w_qr, in_=wqr[:])
                nc.sync.dma_start(out=ones128, in_=ones_d[:])
                if with_biases:
                    nc.sync.dma_start(out=bias1, in_=bias1_d[:])
                    nc.sync.dma_start(out=bias2, in_=bias2_d[:])
                    nc.sync.dma_start(out=biasv, in_=biasv_d[:])

            # persistent phase-1 outputs
            ckv_t = [cpool.tile([128, T], BF, tag=f"ckv{j}", name=f"ckv{j}")
                     for j in range(4)]
            cq_t = [cpool.tile([128, T], BF, tag=f"cq{j}", name=f"cq{j}")
                    for j in range(4)]
            kr_pre = cpool.tile([128, T], BF, tag="kr_pre")
            kr_t = cpool.tile([128, T], BF, tag="kr")
            # warm the ACT exp table (one-time ~2.7us load) off the
            # critical path, before attention needs it
            warm = small.tile([1, 1], F32, tag="warm")
            nc.vector.memset(warm, 0.0)
            nc.scalar.activation(out=warm, in_=warm,
                                 func=mybir.ActivationFunctionType.Exp)

            def evict(dst_ap, src_psum, bias_ap):
                if bias_ap is not None:
                    nc.vector.tensor_scalar_add(out=dst_ap, in0=src_psum,
                                                scalar1=bias_ap)
                else:
                    nc.vector.tensor_copy(out=dst_ap, in_=src_psum)

            # ===== phases 1-2: k-outer projections, 4 chunk accumulators ====
            # (PSUM pool closed before phase 3; PSUM/SBUF pool stacks are
            # independent, so this interleaves fine with the SBUF pools.)
            prps_cm = tc.tile_pool(name="prps", bufs=6, space="PSUM")
            prps = prps_cm.__enter__()
            if True:

                def proj(lhs_tiles, src_tiles, dst, b_ap, tag):
                    nkt = len(lhs_tiles)
                    pss = [prps.tile([128, 512], F32, tag="prps",
                                     name=f"{tag}ps{ch}") for ch in range(NCH)]
                    for kt in range(nkt):
                        for ch in range(NCH):
                            nc.tensor.matmul(
                                pss[ch],
                                lhsT=lhs_tiles[kt],
                                rhs=src_tiles[kt][:, ch * 512: (ch + 1) * 512],
                                start=(kt == 0),
                                stop=(kt == nkt - 1),
                            )
                    for ch in range(NCH):
                        evict(dst[:, ch * 512: (ch + 1) * 512], pss[ch], b_ap)

                def apply_rope(pre_tile, dsts):
                    """out = pre*cos + swap(pre)*sin_signed.
                    dsts: [(tile, row_slice)] destinations for the final add."""
                    for ch in range(NCH):
                        sl = slice(ch * 512, (ch + 1) * 512)
                        sw = prps.tile([128, 512], F32, tag="prps",
                                       name="swpsum")
                        nc.tensor.matmul(sw, lhsT=swp_t[:], rhs=pre_tile[:, sl],
                                         start=True, stop=True)
                        t1 = opool.tile([128, 512], F32, tag="rope_t1")
                        nc.vector.tensor_mul(out=t1, in0=pre_tile[:, sl],
                                             in1=cos_t[:, sl])
                        t2 = opool.tile([128, 512], F32, tag="rope_t2")
                        nc.vector.tensor_mul(out=t2, in0=sw, in1=sin_t[:, sl])
                        for dst, rp in dsts:
                            nc.vector.tensor_add(out=dst[rp, sl],
                                                 in0=t1[rp, :], in1=t2[rp, :])

                # ---- PHASE 1: x^T consumers (x + streamed W resident) ----
                with tc.tile_pool(name="xpool", bufs=1) as xpool, \
                     tc.tile_pool(name="wstream", bufs=2) as wstream:
                    wt0 = wstream.tile([128, NKT_DM * 128], BF, tag="wt",
                                       name="wt0")
                    nc.scalar.dma_start(out=wt0, in_=wdown[0])
                    x_tiles = []
                    for kt in range(NKT_DM):
                        xtile = xpool.tile([128, T], BF, tag=f"x{kt}",
                                           name=f"x{kt}")
                        nc.sync.dma_start(out=xtile, in_=xt[kt])
                        x_tiles.append(xtile)
                    dma_consts()

                    for j in range(9):
                        if j == 0:
                            wt = wt0
                        else:
                            wt = wstream.tile([128, NKT_DM * 128], BF,
                                              tag="wt", name=f"wt{j}")
                            nc.scalar.dma_start(out=wt, in_=wdown[j])
                        if j < 4:
                            dst = ckv_t[j]
                            b_ap = bias1[:, j:j + 1] if with_biases else None
                        elif j < 8:
                            dst = cq_t[j - 4]
                            b_ap = bias1[:, j:j + 1] if with_biases else None
                        else:
                            dst = kr_pre
                            b_ap = bias1[:, 8:9] if with_biases else None
                        proj([wt[:, kt * 128: (kt + 1) * 128]
                              for kt in range(NKT_DM)],
                             x_tiles, dst, b_ap, f"p1j{j}")

                apply_rope(kr_pre, [(kr_t, slice(0, 128))])

                # ---- PHASE 2+3: latent consumers (reuse x's SBUF) ----
                with tc.tile_pool(name="kqpool", bufs=1) as kqpool, \
                     tc.tile_pool(name="vpool", bufs=1) as vpool:
                    kc_t = [kqpool.tile([128, T], BF, tag=f"kc{h}",
                                        name=f"kc{h}") for h in range(HL)]
                    qc_t = [kqpool.tile([128, T], BF, tag=f"qc{h}",
                                        name=f"qc{h}") for h in range(HL)]
                    qr_pre = kqpool.tile([128, T], BF, tag="qr_pre")
                    # per-head zero-padded rope Q: only rows 32h..32h+32 live,
                    # so the rope S matmul is a plain full-128 (0,0) matmul
                    qr_pad = [kqpool.tile([128, T], BF, tag=f"qrp{h}",
                                          name=f"qrp{h}") for h in range(HL)]
                    for h in range(HL):
                        nc.gpsimd.memset(qr_pad[h][:], 0.0)

                    for h in range(HL):
                        proj([w_uk[:, kt * 512 + 128 * h:
                                   kt * 512 + 128 * (h + 1)]
                              for kt in range(NKT_DC)],
                             ckv_t, kc_t[h],
                             bias2[:, h:h + 1] if with_biases else None,
                             f"p2k{h}")
                    for h in range(HL):
                        proj([w_uq[:, kt * 512 + 128 * h:
                                   kt * 512 + 128 * (h + 1)]
                              for kt in range(NKT_DC)],
                             cq_t, qc_t[h],
                             bias2[:, 4 + h:5 + h] if with_biases else None,
                             f"p2q{h}")
                    proj([w_qr[:, kt * 128: (kt + 1) * 128]
                          for kt in range(NKT_DC)],
                         cq_t, qr_pre,
                         bias2[:, 8:9] if with_biases else None, "p2r")

                    apply_rope(qr_pre, [(qr_pad[h], slice(32 * h, 32 * h + 32))
                                        for h in range(HL)])

                    # V natural: [T-tile rows, 512 (4 heads x 128)]
                    v_nat = []
                    for tt in range(NTT):
                        ps = prps.tile([128, 512], F32, tag="prps",
                                       name=f"vps{tt}")
                        for kt in range(NKT_DC):
                            nc.tensor.matmul(
                                ps,
                                lhsT=ckv_t[kt][:, tt * 128: (tt + 1) * 128],
                                rhs=w_uv[:, kt * 512: (kt + 1) * 512],
                                start=(kt == 0),
                                stop=(kt == NKT_DC - 1),
                            )
                        vt = vpool.tile([128, 512], BF, tag=f"v{tt}",
                                        name=f"v{tt}")
                        if with_biases:
                            nc.vector.tensor_add(out=vt, in0=ps, in1=biasv)
                        else:
                            nc.vector.tensor_copy(out=vt, in_=ps)
                        v_nat.append(vt)

                    # ========== PHASE 3: attention ==========
                    prps_cm.__exit__(None, None, None)
                    with tc.tile_pool(name="sp2", bufs=2, space="PSUM") as sp2, \
                         tc.tile_pool(name="opp", bufs=2,
                                      space="PSUM") as op_pool, \
                         tc.tile_pool(name="lpp", bufs=2,
                                      space="PSUM") as lp_pool:
                        for h in range(HL):
                            for qc in range(NCH):
                                qsl = slice(qc * 512, (qc + 1) * 512)
                                # all-ones lhsT: every row of lacc = l[q]
                                # (full-128 matmul + free broadcast)
                                lacc = lp_pool.tile([128, 512], F32,
                                                    tag="lacc",
                                                    name=f"lacc{h}_{qc}")
                                outp = op_pool.tile([128, 512], F32,
                                                    tag="outp",
                                                    name=f"outp{h}_{qc}")
                                # software pipeline: PV/l of pair kp run
                                # after S of pair kp+1, so the PE never waits
                                # on the ACT exp.
                                pts = [None] * (NTT // 2)

                                def pv_l(kp):
                                    for ki in range(2):
                                        kt = 2 * kp + ki
                                        psl = slice(512 * ki, 512 * (ki + 1))
                                        nc.tensor.matmul(
                                            outp,
                                            lhsT=v_nat[kt][:, 128 * h:
                                                           128 * (h + 1)],
                                            rhs=pts[kp][:, psl],
                                            start=(kt == 0),
                                            stop=(kt == NTT - 1))
                                        nc.tensor.matmul(
                                            lacc, lhsT=ones128[:],
                                            rhs=pts[kp][:, psl],
                                            start=(kt == 0),
                                            stop=(kt == NTT - 1))

                                for kp in range(NTT // 2):
                                    spt = sp2.tile([128, 1024], F32, tag="sp",
                                                   name=f"sp{h}_{qc}_{kp}")
                                    for ki in range(2):
                                        kt = 2 * kp + ki
                                        ksl = slice(kt * 128, (kt + 1) * 128)
                                        half = spt[:, 512 * ki: 512 * (ki + 1)]
                                        nc.tensor.matmul(
                                            half, lhsT=kc_t[h][:, ksl],
                                            rhs=qc_t[h][:, qsl],
                                            start=True, stop=False)
                                        nc.tensor.matmul(
                                            half, lhsT=kr_t[:, ksl],
                                            rhs=qr_pad[h][:, qsl],
                                            start=False, stop=True)
                                    pt = ppool.tile([128, 1024], BF, tag="pt")
                                    nc.scalar.activation(
                                        out=pt, in_=spt,
                                        func=mybir.ActivationFunctionType.Exp,
                                        scale=SCALE)
                                    pts[kp] = pt
                                    if kp > 0:
                                        pv_l(kp - 1)
                                pv_l(NTT // 2 - 1)
                                # normalize: out = outp * (1/l)
                                rinv = small.tile([128, 512], F32, tag="rinv")
                                nc.vector.reciprocal(out=rinv, in_=lacc)
                                o_sb = opool.tile([128, 512], F32, tag="o_sb")
                                nc.vector.tensor_mul(out=o_sb, in0=outp,
                                                     in1=rinv)
                                nc.sync.dma_start(
                                    out=out_t[128 * h: 128 * (h + 1), qsl],
                                    in_=o_sb)

    _split_multi_waits(nc)
    return nc


_nc_cache = {}


def _get_nc(with_biases):
    if with_biases not in _nc_cache:
        _nc_cache[with_biases] = _build_nc(with_biases)
    return _nc_cache[with_biases]


def _rope_perm():
    """Permutation of the 32 rope dims within one head: evens then odds."""
    return np.concatenate([np.arange(0, DR, 2), np.arange(1, DR, 2)])


def kernel(x, W_DKV, b_DKV, W_UK, b_UK, W_UV, b_UV, W_DQ, b_DQ,
           W_UQ, b_UQ, W_QR, b_QR, W_KR, b_KR):
    x = np.asarray(x, np.float32)
    f32 = lambda a: np.asarray(a, np.float32)
    W_DKV, W_UK, W_UV, W_DQ, W_UQ, W_QR, W_KR = map(
        f32, (W_DKV, W_UK, W_UV, W_DQ, W_UQ, W_QR, W_KR))
    b_DKV, b_UK, b_UV, b_DQ, b_UQ, b_QR, b_KR = map(
        f32, (b_DKV, b_UK, b_UV, b_DQ, b_UQ, b_QR, b_KR))

    with_biases = any(np.any(b)
                      for b in (b_DKV, b_UK, b_UV, b_DQ, b_UQ, b_QR, b_KR))
    nc = _get_nc(with_biases)

    perm = _rope_perm()

    # lhsT-tile layout helper: W [K, C] -> [128, (K//128)*C], [p, kt*C + c]
    def tile_k(w):
        k, c = w.shape
        return np.ascontiguousarray(
            w.reshape(k // 128, 128, c).transpose(1, 0, 2).reshape(128, -1)
        ).astype(BF16)

    # column-block layout for streamed down-proj weights:
    # W [2048, C] -> per 128-col block j: [128, 16*128], [p, kt*128 + cc]
    def tile_k_blocks(w):
        k, c = w.shape
        nj = c // 128
        return np.ascontiguousarray(
            w.reshape(k // 128, 128, nj, 128).transpose(2, 1, 0, 3)
            .reshape(nj, 128, -1)
        ).astype(BF16)

    # x^T per batch, tiled over d_model: [16, 128, T]
    xt_b = []
    for b in range(B):
        xT = np.ascontiguousarray(x[b].T.astype(BF16))       # [DM, T]
        xt_b.append(np.ascontiguousarray(xT.reshape(NKT_DM, 128, T)))

    # RoPE tables: [128, T] fp32; rows 32h+i / 32h+16+i use freq i
    freqs = 10000.0 ** (-(np.arange(0, DR, 2, dtype=np.float64) / DR))   # [16]
    theta = np.arange(T, dtype=np.float64)[:, None] * freqs[None, :]     # [T, 16]
    cos16 = np.cos(theta).T.astype(np.float32)                           # [16, T]
    sin16 = np.sin(theta).T.astype(np.float32)
    cos_full = np.tile(cos16, (8, 1))                                    # [128, T]
    sin_signed = np.tile(np.concatenate([-sin16, sin16], 0), (4, 1))     # [128, T]

    # swap permutation matrix (16-row block swap inside each 32-row block)
    swp = np.zeros((128, 128), np.float32)
    for hb in range(4):
        for i in range(16):
            swp[32 * hb + 16 + i, 32 * hb + i] = 1.0
            swp[32 * hb + i, 32 * hb + 16 + i] = 1.0
    swp = swp.astype(BF16)

    ones128 = np.ones((128, 128), BF16)

    in_maps = []
    for c in range(8):
        b, g = divmod(c, 4)
        heads = slice(4 * g * DH, (4 * g + HL) * DH)          # content cols
        rcols = np.concatenate(
            [(4 * g + h) * DR + perm for h in range(HL)])     # rope cols
        wdown = np.concatenate([
            tile_k_blocks(W_DKV),            # j=0..3
            tile_k_blocks(W_DQ),             # j=4..7
            tile_k_blocks(W_KR[:, rcols]),   # j=8
        ], axis=0)
        RSTRIDE = 4 * 128 * 512
        JSTRIDE = 128 * 512
        m = {
            "xt": xt_b[b],
            "xs": np.ascontiguousarray(
                xt_b[b][:, :, 512 * g: 512 * (g + 1)]),
            "rbase": np.array([[4 * b * RSTRIDE + (j % 4) * JSTRIDE
                                for j in range(8)]], np.uint32),
            "wdown": wdown,
            "wuk": tile_k(W_UK[:, heads]),
            "wuv": tile_k(W_UV[:, heads]),
            "wuq": tile_k(W_UQ[:, heads]),
            "wqr": tile_k(W_QR[:, rcols]),
            "cos": cos_full,
            "sin": sin_signed,
            "swp": swp,
            "ones128": ones128,
        }
        if with_biases:
            bias1 = np.zeros((128, 9), np.float32)
            bias1[:, 0:4] = b_DKV.reshape(4, 128).T
            bias1[:, 4:8] = b_DQ.reshape(4, 128).T
            bias1[:, 8] = b_KR[rcols]
            bias2 = np.zeros((128, 9), np.float32)
            bias2[:, 0:4] = b_UK[heads].reshape(4, 128).T
            bias2[:, 4:8] = b_UQ[heads].reshape(4, 128).T
            bias2[:, 8] = b_QR[rcols]
            m["bias1"] = bias1
            m["bias2"] = bias2
            m["biasv"] = np.tile(b_UV[heads][None, :], (128, 1)).astype(np.float32)
        in_maps.append(m)

    res = run_bass_kernel_spmd(nc, in_maps, core_ids=list(range(8)))

    out = np.empty((B, T, DM), np.float32)
    for c in range(8):
        b, g = divmod(c, 4)
        ot = res.results[c]["out_t"]                    # [512, T]
        for h in range(HL):
            out[b, :, (4 * g + h) * DH: (4 * g + h + 1) * DH] = \
                ot[128 * h: 128 * (h + 1), :].T
    return out
